# revision 9
# baseline (speedup 1.0000x reference)
"""Trainium2 Bass kernel for nn_BSquareModelCombined (histogram_binning).

Model: votes = scatter_add(relu(relu(x@W1+b1)@W2+b2) @ Wout + bout)
At RATIO=0 the vote mask is always true, so the final scatter is a fixed
linear map S [992 -> 32]; we fold Wout@S and bout@S on the host and the
device computes a 3-layer chain of matmuls.

Sharding (8 cores): fc1 replicated (each core computes the full h1 from x
and W1), fc2 column-sharded (each core owns 1984 of the 15872 output
columns of W2 — a single aggregate read of the 1 GB W2), fc_out
row-sharded (partial [32, 256] vote sums per core, reduced on the host).
All matmuls run in bf16 with fp32 PSUM accumulation; activations
(ReLU + bias) run on the scalar engine in fp32.

Layout: all activations live transposed on-chip (batch on the free dim,
feature on partitions): xT -> h1T -> h2T -> votesT. Every matmul is
out[feat_tile, batch] = W_tile.T @ actT, contraction on partitions.

Self-contained: only needs /opt/trn_rl_repo (present in the container).
"""
import sys

sys.path.insert(0, "/opt/trn_rl_repo")

from contextlib import ExitStack

import numpy as np
import ml_dtypes

import jax
from jax.sharding import Mesh, PartitionSpec
from jax.experimental.shard_map import shard_map

import concourse.bass as bass  # noqa: F401  (registers AP machinery)
import concourse.tile as tile
import concourse.mybir as mybir
from concourse import bacc, bass2jax

# ---------------------------------------------------------------- problem dims
P = 128
B = 256            # batch
IN = 512           # fc1 input dim
HID = 15872        # hidden dim (= 124 * 128)
NUM_CLASSES = 32
TRI = NUM_CLASSES * (NUM_CLASSES - 1) // 2  # 496
N_CORES = 8
SHARD = HID // N_CORES   # 1984 fc2 output cols per core
KO1 = IN // P            # 4 contraction chunks in fc1
NT1 = HID // P           # 124 fc1 output tiles (128 wide)
M2 = 124                 # fc2 output tile width (16 * 124 = 1984)
NT2 = SHARD // M2        # 16 fc2 output tiles per core
NK2 = HID // P           # 124 fc2 contraction tiles
SLABS = [4] + [8] * 15   # fc1 weight slab sizes in m-tiles (sum = 124)
W1_BUFS = 4

# fp16 over bf16: same 2-byte DMA cost and same 1-cycle/row matmul rate,
# but 10 mantissa bits instead of 7 (~8x lower rounding error). All values
# here are O(1-100), far inside fp16 range.
BF = mybir.dt.float16
F32 = mybir.dt.float32
RELU = mybir.ActivationFunctionType.Relu


def build_nc(reps: int = 1, w2_bufs: int = 6, timing: bool = False,
             slabs=None, w1_bufs=None):
    """Build the per-core SPMD Bass program. reps>1 wraps the body in a
    hardware loop; timing=True makes the big weights Internal DRAM
    (uninitialized — contents irrelevant for wall-clock measurement) so a
    timed call only transfers the small tensors."""
    nc = bacc.Bacc(
        "TRN2",
        target_bir_lowering=False,
        debug=False,
        enable_asserts=False,
        num_devices=N_CORES,
    )
    slabs = SLABS if slabs is None else slabs
    w1_bufs = W1_BUFS if w1_bufs is None else w1_bufs
    assert sum(slabs) == NT1
    big = (lambda n, s, d: nc.dram_tensor(n, s, d).ap()) if timing else (
        lambda n, s, d: nc.dram_tensor(n, s, d, kind="ExternalInput").ap())
    xT = nc.dram_tensor("xt", [IN, B], BF, kind="ExternalInput").ap()
    w1 = big("w1", [IN, HID], BF)
    b1t = nc.dram_tensor("b1t", [P, NT1], F32, kind="ExternalInput").ap()
    w2 = big("w2", [HID, SHARD], BF)
    b2t = nc.dram_tensor("b2t", [M2, NT2], F32, kind="ExternalInput").ap()
    wf = big("wf", [SHARD, NUM_CLASSES], BF)
    votes = nc.dram_tensor("votes", [NUM_CLASSES, B], F32, kind="ExternalOutput").ap()

    w1r = w1.rearrange("(ko ki) m -> ki ko m", ki=P)
    wfr = wf.rearrange("(t p) c -> p t c", p=M2)

    with tile.TileContext(nc) as tc, ExitStack() as ctx:
        consts = ctx.enter_context(tc.tile_pool(name="consts", bufs=1))
        w1pool = ctx.enter_context(tc.tile_pool(name="w1p", bufs=w1_bufs))
        w2pool = ctx.enter_context(tc.tile_pool(name="w2p", bufs=w2_bufs))
        h1pool = ctx.enter_context(tc.tile_pool(name="h1p", bufs=1))
        h2pool = ctx.enter_context(tc.tile_pool(name="h2p", bufs=1))
        outpool = ctx.enter_context(tc.tile_pool(name="outp", bufs=2))
        psum = ctx.enter_context(tc.tile_pool(name="psum", bufs=8, space="PSUM"))

        def body():
            # ---- constants
            xtsb = consts.tile([P, KO1, B], BF, name="xtsb")
            nc.sync.dma_start(xtsb[:], xT.rearrange("(ko ki) b -> ki ko b", ki=P))
            b1sb = consts.tile([P, NT1], F32, name="b1sb")
            nc.sync.dma_start(b1sb[:], b1t[:])
            h1sb = h1pool.tile([P, NT1, B], BF, name="h1sb")

            def relu_bias(dst, src_ps, bias_ap, use_dve):
                # out = relu(src + bias); alternate engines so neither the
                # scalar (ACT) nor vector (DVE) engine gates PSUM recycling
                if use_dve:
                    nc.vector.tensor_scalar(
                        dst, src_ps, bias_ap, 0.0,
                        mybir.AluOpType.add, mybir.AluOpType.max,
                    )
                else:
                    nc.scalar.activation(dst, src_ps, RELU, bias=bias_ap)

            # ---- fc1: h1T[m, b] = relu(W1.T @ xT + b1); W1 streamed in
            # small slabs (first one tiny to cut the startup bubble)
            mt = 0
            for s, ntiles in enumerate(slabs):
                w1sb = w1pool.tile(
                    [P, KO1, max(slabs) * P], BF, name="w1sb"
                )
                nc.sync.dma_start(
                    w1sb[:, :, :ntiles * P],
                    w1r[:, :, mt * P:(mt + ntiles) * P],
                )
                for t in range(ntiles):
                    ps = psum.tile([P, 512], F32, name="ps", tag="ps")
                    for ko in range(KO1):
                        nc.tensor.matmul(
                            ps[:, :B],
                            w1sb[:, ko, t * P:(t + 1) * P],
                            xtsb[:, ko, :],
                            start=(ko == 0),
                            stop=(ko == KO1 - 1),
                        )
                    relu_bias(
                        h1sb[:, mt, :], ps[:, :B], b1sb[:, mt:mt + 1],
                        use_dve=(mt % 2 == 1),
                    )
                    mt += 1

            # ---- fc2: 16 PSUM-resident accumulators (2 per bank), single
            # pass over W2's rows. Only the even half of each bank issues
            # start=True (a start clears the WHOLE bank); the odd half's
            # first matmul overwrites via the per-element has_written bit.
            # tail constants load during fc2's long DMA stream
            b2sb = consts.tile([M2, NT2], F32, name="b2sb")
            nc.sync.dma_start(b2sb[:], b2t[:])
            wfsb = consts.tile([M2, NT2, NUM_CLASSES], BF, name="wfsb")
            nc.sync.dma_start(wfsb[:], wfr[:])

            accs = [
                psum.tile([P, 512], F32, name=f"acc{j}", tag="ps")
                for j in range(NT2 // 2)
            ]
            h2sb = h2pool.tile([M2, NT2, B], BF, name="h2sb")
            for k in range(NK2):
                w2sb = w2pool.tile([P, SHARD], BF, name="w2sb")
                nc.sync.dma_start(w2sb[:], w2[k * P:(k + 1) * P, :])
                last = k == NK2 - 1
                for mt in range(NT2):
                    j, half = divmod(mt, 2)
                    nc.tensor.matmul(
                        accs[j][:M2, half * B:(half + 1) * B],
                        w2sb[:, mt * M2:(mt + 1) * M2],
                        h1sb[:, k, :],
                        start=(k == 0 and half == 0),
                        stop=last,
                        skip_group_check=True,
                    )
                    if last and half == 1:
                        # bank j complete: relu both halves now so fc_out
                        # overlaps the remaining banks' final matmuls
                        for m2 in (mt - 1, mt):
                            relu_bias(
                                h2sb[:, m2, :],
                                accs[j][:M2, (m2 % 2) * B:(m2 % 2 + 1) * B],
                                b2sb[:, m2:m2 + 1],
                                use_dve=(m2 % 2 == 1),
                            )

            # ---- fc_out partial votes
            vps = psum.tile([NUM_CLASSES, 512], F32, name="vps", tag="ps")
            for mt in range(NT2):
                nc.tensor.matmul(
                    vps[:NUM_CLASSES, :B],
                    wfsb[:, mt, :],
                    h2sb[:, mt, :],
                    start=(mt == 0),
                    stop=(mt == NT2 - 1),
                )
            vsb = outpool.tile([NUM_CLASSES, B], F32, name="vsb")
            nc.vector.tensor_copy(vsb[:], vps[:NUM_CLASSES, :B])
            nc.sync.dma_start(votes[:], vsb[:])

        if reps == 1:
            body()
        else:
            with tc.For_i(0, reps, 1):
                body()

    nc.compile()
    return nc


# ------------------------------------------------------------------ PJRT runner
class SpmdRunner:
    """Cached-jit SPMD executor (mirrors bass2jax.run_bass_via_pjrt)."""

    def __init__(self, nc, n_cores: int):
        bass2jax.install_neuronx_cc_hook()
        self.nc = nc
        self.n_cores = n_cores
        partition_name = (
            nc.partition_id_tensor.name if nc.partition_id_tensor else None
        )

        in_names, out_names, out_avals, zero_shapes = [], [], [], []
        for alloc in nc.m.functions[0].allocations:
            if not isinstance(alloc, mybir.MemoryLocationSet):
                continue
            name = alloc.memorylocations[0].name
            if alloc.kind == "ExternalInput":
                if name != partition_name:
                    in_names.append(name)
            elif alloc.kind == "ExternalOutput":
                out_names.append(name)
                shape = tuple(alloc.tensor_shape)
                dtype = mybir.dt.np(alloc.dtype)
                out_avals.append(jax.core.ShapedArray(shape, dtype))
                zero_shapes.append((shape, dtype))

        self.in_names = in_names
        self.out_names = out_names
        self.out_avals = out_avals
        self.zero_shapes = zero_shapes
        n_params = len(in_names)
        n_outs = len(out_avals)
        all_in_names = in_names + out_names
        if partition_name is not None:
            all_in_names.append(partition_name)
        donate = tuple(range(n_params, n_params + n_outs))

        def _body(*args):
            operands = list(args)
            if partition_name is not None:
                operands.append(bass2jax.partition_id_tensor())
            outs = bass2jax._bass_exec_p.bind(
                *operands,
                out_avals=tuple(out_avals),
                in_names=tuple(all_in_names),
                out_names=tuple(out_names),
                lowering_input_output_aliases=(),
                sim_require_finite=True,
                sim_require_nnan=True,
                nc=nc,
            )
            return tuple(outs)

        devices = jax.devices()[:n_cores]
        assert len(devices) == n_cores
        mesh = Mesh(np.asarray(devices), ("core",))
        self.sharded = jax.jit(
            shard_map(
                _body,
                mesh=mesh,
                in_specs=(PartitionSpec("core"),) * (n_params + n_outs),
                out_specs=(PartitionSpec("core"),) * n_outs,
                check_rep=False,
            ),
            donate_argnums=donate,
            keep_unused=True,
        )

    def concat_inputs(self, in_maps):
        return [
            np.concatenate(
                [np.asarray(m[nm]) for m in in_maps], axis=0
            )
            for nm in self.in_names
        ]

    def __call__(self, concat_in):
        zeros = [
            np.zeros((self.n_cores * s[0], *s[1:]), d)
            for (s, d) in self.zero_shapes
        ]
        out_arrs = self.sharded(*concat_in, *zeros)
        return [
            {
                nm: np.asarray(out_arrs[i]).reshape(
                    self.n_cores, *self.out_avals[i].shape
                )[c]
                for i, nm in enumerate(self.out_names)
            }
            for c in range(self.n_cores)
        ]


# --------------------------------------------------------------- host plumbing
_I_IDX, _J_IDX = np.triu_indices(NUM_CLASSES, k=1)


def fold_scatter(Wout: np.ndarray, bout: np.ndarray):
    """Fold the always-true vote scatter into the output weights:
    votes = h2 @ Wfold + bfold with Wfold[:, c] = sum of Wout cols voting
    for class c."""
    Wfold = np.zeros((HID, NUM_CLASSES), np.float32)
    bfold = np.zeros(NUM_CLASSES, np.float32)
    for c in range(NUM_CLASSES):
        ci = np.where(_I_IDX == c)[0] * 2
        cj = np.where(_J_IDX == c)[0] * 2 + 1
        cols = np.concatenate([ci, cj])
        Wfold[:, c] = Wout[:, cols].sum(axis=1)
        bfold[c] = bout[cols].sum()
    return Wfold, bfold


def make_in_maps(x, W1, b1, W2, b2, Wout, bout):
    bfl = np.float16
    x = np.asarray(x, np.float32)
    W1 = np.asarray(W1, np.float32)
    b1 = np.asarray(b1, np.float32)
    W2 = np.asarray(W2, np.float32)
    b2 = np.asarray(b2, np.float32)
    Wout = np.asarray(Wout, np.float32)
    bout = np.asarray(bout, np.float32)

    Wfold, bfold = fold_scatter(Wout, bout)

    xt = np.ascontiguousarray(x.T).astype(bfl)
    w1b = W1.astype(bfl)
    b1tt = np.ascontiguousarray(b1.reshape(NT1, P).T)
    W2b = W2.astype(bfl)
    Wfb = Wfold.astype(bfl)

    in_maps = []
    for c in range(N_CORES):
        lo, hi = c * SHARD, (c + 1) * SHARD
        in_maps.append({
            "xt": xt,
            "w1": w1b,
            "b1t": b1tt,
            "w2": np.ascontiguousarray(W2b[:, lo:hi]),
            "b2t": np.ascontiguousarray(b2[lo:hi].reshape(NT2, M2).T),
            "wf": np.ascontiguousarray(Wfb[lo:hi, :]),
        })
    return in_maps, bfold


_RUNNER_CACHE = {}


def get_runner(reps: int = 1):
    if reps not in _RUNNER_CACHE:
        nc = build_nc(reps=reps)
        _RUNNER_CACHE[reps] = SpmdRunner(nc, N_CORES)
    return _RUNNER_CACHE[reps]


def kernel(x, W1, b1, W2, b2, Wout, bout):
    in_maps, bfold = make_in_maps(x, W1, b1, W2, b2, Wout, bout)
    runner = get_runner(reps=1)
    results = runner(runner.concat_inputs(in_maps))
    votes_t = np.zeros((NUM_CLASSES, B), np.float32)
    for c in range(N_CORES):
        votes_t += results[c]["votes"]
    return votes_t.T + bfold[None, :]


# ------------------------------------------------------- V2: fc1 all-gathered
HID_PAD = 16384          # pad hidden to 128 tiles so each core owns 16
NTG = HID_PAD // P       # 128 global m-tiles
LOC = NTG // N_CORES     # 16 local tiles per core
GCH = 4                  # gather chunks (4 tiles each)
CPT = LOC // GCH         # tiles per chunk

# fc2 consumption order: chunk-major so gathered tiles arrive before use
KS_V2 = [
    c * LOC + t
    for g in range(GCH)
    for c in range(N_CORES)
    for t in range(g * CPT, (g + 1) * CPT)
    if c * LOC + t < NK2
]
assert len(KS_V2) == NK2


def build_nc_v2(reps: int = 1, w2_bufs: int = 6, timing: bool = False,
                loop_gather: bool = False):
    """fc1 sharded 8-way + chunked AllGather of h1 (bf16, via shared DRAM).

    reps>1 requires loop_gather=False: collectives cannot live inside a
    hardware loop on this runtime, so the timing build hoists the gathers
    out of the loop (the looped body still pays the spill + readback DMA,
    but not the link transfer)."""
    nc = bacc.Bacc(
        "TRN2",
        target_bir_lowering=False,
        debug=False,
        enable_asserts=False,
        num_devices=N_CORES,
    )
    big = (lambda n, s, d: nc.dram_tensor(n, s, d).ap()) if timing else (
        lambda n, s, d: nc.dram_tensor(n, s, d, kind="ExternalInput").ap())
    xT = nc.dram_tensor("xt", [IN, B], BF, kind="ExternalInput").ap()
    w1s = big("w1s", [IN, LOC * P], BF)           # this core's W1 slice
    b1ts = nc.dram_tensor("b1ts", [P, LOC], F32, kind="ExternalInput").ap()
    w2 = big("w2", [HID, SHARD], BF)
    b2t = nc.dram_tensor("b2t", [M2, NT2], F32, kind="ExternalInput").ap()
    wf = big("wf", [SHARD, NUM_CLASSES], BF)
    votes = nc.dram_tensor("votes", [NUM_CLASSES, B], F32, kind="ExternalOutput").ap()

    cc_in = [nc.dram_tensor(f"cci{g}", [CPT, P, B], BF).ap() for g in range(GCH)]
    cc_out = [
        nc.dram_tensor(f"cco{g}", [N_CORES, CPT, P, B], BF, addr_space="Shared").ap()
        for g in range(GCH)
    ]

    w1r = w1s.rearrange("(ko ki) m -> ki ko m", ki=P)
    wfr = wf.rearrange("(t p) c -> p t c", p=M2)

    with tile.TileContext(nc) as tc, ExitStack() as ctx:
        consts = ctx.enter_context(tc.tile_pool(name="consts", bufs=1))
        w1pool = ctx.enter_context(tc.tile_pool(name="w1p", bufs=2))
        w2pool = ctx.enter_context(tc.tile_pool(name="w2p", bufs=w2_bufs))
        h1loc_p = ctx.enter_context(tc.tile_pool(name="h1lp", bufs=1))
        h1pool = ctx.enter_context(tc.tile_pool(name="h1p", bufs=1))
        h2pool = ctx.enter_context(tc.tile_pool(name="h2p", bufs=1))
        outpool = ctx.enter_context(tc.tile_pool(name="outp", bufs=2))
        psum = ctx.enter_context(tc.tile_pool(name="psum", bufs=8, space="PSUM"))

        def gather(g):
            nc.gpsimd.collective_compute(
                "AllGather",
                mybir.AluOpType.bypass,
                replica_groups=[list(range(N_CORES))],
                ins=[cc_in[g][:]],
                outs=[cc_out[g][:]],
            )

        def body(do_gather=True):
            xtsb = consts.tile([P, KO1, B], BF, name="xtsb")
            nc.sync.dma_start(xtsb[:], xT.rearrange("(ko ki) b -> ki ko b", ki=P))
            b1sb = consts.tile([P, LOC], F32, name="b1sb")
            nc.sync.dma_start(b1sb[:], b1ts[:])

            def relu_bias(dst, src_ps, bias_ap, use_dve):
                if use_dve:
                    nc.vector.tensor_scalar(
                        dst, src_ps, bias_ap, 0.0,
                        mybir.AluOpType.add, mybir.AluOpType.max,
                    )
                else:
                    nc.scalar.activation(dst, src_ps, RELU, bias=bias_ap)

            # fc1 on the local 16 tiles, spilled to DRAM per 4-tile chunk
            h1loc = h1loc_p.tile([P, LOC, B], BF, name="h1loc")
            w1sb = w1pool.tile([P, KO1, LOC * P], BF, name="w1sb")
            nc.sync.dma_start(w1sb[:], w1r[:])
            for t in range(LOC):
                ps = psum.tile([P, 512], F32, name="ps", tag="ps")
                for ko in range(KO1):
                    nc.tensor.matmul(
                        ps[:, :B],
                        w1sb[:, ko, t * P:(t + 1) * P],
                        xtsb[:, ko, :],
                        start=(ko == 0),
                        stop=(ko == KO1 - 1),
                    )
                relu_bias(h1loc[:, t, :], ps[:, :B], b1sb[:, t:t + 1],
                          use_dve=(t % 2 == 1))
                if t % CPT == CPT - 1:
                    g = t // CPT
                    nc.sync.dma_start(
                        cc_in[g].rearrange("t p b -> p t b"),
                        h1loc[:, g * CPT:(g + 1) * CPT, :],
                    )
                    if do_gather:
                        gather(g)

            # gathered h1 (padded to 128 tiles; tiles >= 124 unused)
            h1sb = h1pool.tile([P, NTG, B], BF, name="h1sb")
            for g in range(GCH):
                for r in range(N_CORES):
                    nc.sync.dma_start(
                        h1sb[:, r * LOC + g * CPT:r * LOC + (g + 1) * CPT, :],
                        cc_out[g][r].rearrange("t p b -> p t b"),
                    )

            # tail constants during fc2's stream
            b2sb = consts.tile([M2, NT2], F32, name="b2sb")
            nc.sync.dma_start(b2sb[:], b2t[:])
            wfsb = consts.tile([M2, NT2, NUM_CLASSES], BF, name="wfsb")
            nc.sync.dma_start(wfsb[:], wfr[:])

            accs = [
                psum.tile([P, 512], F32, name=f"acc{j}", tag="ps")
                for j in range(NT2 // 2)
            ]
            h2sb = h2pool.tile([M2, NT2, B], BF, name="h2sb")
            for i, k in enumerate(KS_V2):
                w2sb = w2pool.tile([P, SHARD], BF, name="w2sb")
                nc.sync.dma_start(w2sb[:], w2[k * P:(k + 1) * P, :])
                first = i == 0
                last = i == NK2 - 1
                for mt in range(NT2):
                    j, half = divmod(mt, 2)
                    nc.tensor.matmul(
                        accs[j][:M2, half * B:(half + 1) * B],
                        w2sb[:, mt * M2:(mt + 1) * M2],
                        h1sb[:, k, :],
                        start=(first and half == 0),
                        stop=last,
                        skip_group_check=True,
                    )
                    if last and half == 1:
                        for m2 in (mt - 1, mt):
                            relu_bias(
                                h2sb[:, m2, :],
                                accs[j][:M2, (m2 % 2) * B:(m2 % 2 + 1) * B],
                                b2sb[:, m2:m2 + 1],
                                use_dve=(m2 % 2 == 1),
                            )

            vps = psum.tile([NUM_CLASSES, 512], F32, name="vps", tag="ps")
            for mt in range(NT2):
                nc.tensor.matmul(
                    vps[:NUM_CLASSES, :B],
                    wfsb[:, mt, :],
                    h2sb[:, mt, :],
                    start=(mt == 0),
                    stop=(mt == NT2 - 1),
                )
            vsb = outpool.tile([NUM_CLASSES, B], F32, name="vsb")
            nc.vector.tensor_copy(vsb[:], vps[:NUM_CLASSES, :B])
            nc.sync.dma_start(votes[:], vsb[:])

        if reps == 1:
            body(do_gather=True)
        else:
            assert not loop_gather
            # collectives can't live in a For_i: gather real data once, then
            # loop the full body minus the link transfer
            zsb = outpool.tile([P, B], BF, name="zsb")
            nc.vector.memset(zsb[:], 0.0)
            for g in range(GCH):
                for t in range(CPT):
                    nc.sync.dma_start(cc_in[g][t], zsb[:])
                gather(g)
            with tc.For_i(0, reps, 1):
                body(do_gather=False)

    nc.compile()
    return nc


def make_in_maps_v2(x, W1, b1, W2, b2, Wout, bout):
    bfl = np.float16
    x = np.asarray(x, np.float32)
    W1 = np.asarray(W1, np.float32)
    b1 = np.asarray(b1, np.float32)
    W2 = np.asarray(W2, np.float32)
    b2 = np.asarray(b2, np.float32)
    Wout = np.asarray(Wout, np.float32)
    bout = np.asarray(bout, np.float32)

    Wfold, bfold = fold_scatter(Wout, bout)

    xt = np.ascontiguousarray(x.T).astype(bfl)
    W1p = np.zeros((IN, HID_PAD), np.float32)
    W1p[:, :HID] = W1
    b1p = np.zeros(HID_PAD, np.float32)
    b1p[:HID] = b1
    W2b = W2.astype(bfl)
    Wfb = Wfold.astype(bfl)

    in_maps = []
    for c in range(N_CORES):
        lo, hi = c * SHARD, (c + 1) * SHARD
        l1, h1_ = c * LOC * P, (c + 1) * LOC * P
        in_maps.append({
            "xt": xt,
            "w1s": np.ascontiguousarray(W1p[:, l1:h1_]).astype(bfl),
            "b1ts": np.ascontiguousarray(b1p[l1:h1_].reshape(LOC, P).T),
            "w2": np.ascontiguousarray(W2b[:, lo:hi]),
            "b2t": np.ascontiguousarray(b2[lo:hi].reshape(NT2, M2).T),
            "wf": np.ascontiguousarray(Wfb[lo:hi, :]),
        })
    return in_maps, bfold


def kernel_v2(x, W1, b1, W2, b2, Wout, bout):
    in_maps, bfold = make_in_maps_v2(x, W1, b1, W2, b2, Wout, bout)
    key = ("v2", 1)
    if key not in _RUNNER_CACHE:
        _RUNNER_CACHE[key] = SpmdRunner(build_nc_v2(reps=1), N_CORES)
    runner = _RUNNER_CACHE[key]
    results = runner(runner.concat_inputs(in_maps))
    votes_t = np.zeros((NUM_CLASSES, B), np.float32)
    for c in range(N_CORES):
        votes_t += results[c]["votes"]
    return votes_t.T + bfold[None, :]
